# revision 6
# baseline (speedup 1.0000x reference)
"""Trainium2 Bass kernel for nn_ContrastiveLoss — v3: symmetry-halved.

The similarity matrix is symmetric, so each off-diagonal superblock pair
(2048x2048 superblocks, 4x4 grid) is computed ONCE; the mirrored
contribution is recovered by column-summing the exp'd tiles.  Row sums
come free via the exp activation's accum_out; column sums are
elementwise-accumulated on the (otherwise idle) DVE into an SBUF f16
tile per column chunk, shipped to the host, and partition-reduced there.

Exp work: 8192*(8192+2048)/2 = 41.9M entries vs 67M direct (63%).

Static SPMD schedule (identical program, per-core data):
  chunk0 cols [   0:2048]: diag rows block0 (16 tiles) + off rows blocks
      1,2 (32 colsum tiles)  -> per core 2 diag + 4 colsum slots
  chunk1 cols [2048:4096]: diag block1 + off blocks 2,3  -> 2 + 4
  chunk2 cols [4096:6144]: diag block2 + off block 3     -> 2 + 2
  chunk3 cols [6144:8192]: diag block3 + off block 0     -> 2 + 2
Each pair {a,b} of superblocks is covered exactly once (rowsum covers
one orientation, colsum the mirror); diagonal superblocks are computed
in full and rowsum-only.  Per core: 20 tile-jobs, identical structure.

Host does normalization, fp8 cast, exact masked-pair corrections and
the numerator, as in v2.
"""

import sys

for _p in ("/opt/trn_rl_repo", "/root/.axon_site/_ro/trn_rl_repo"):
    if _p not in sys.path:
        sys.path.insert(0, _p)

from contextlib import ExitStack

import ml_dtypes
import numpy as np

from concourse import bacc, mybir, tile
from concourse.bass_utils import run_bass_kernel_spmd

F32 = mybir.dt.float32
F16 = mybir.dt.float16
FP8 = mybir.dt.float8e4
FP8NP = ml_dtypes.float8_e4m3

P = 128
B = 4096
D = 256
N = 2 * B                       # 8192
KT = D // P                     # 2 k-tiles, fused via DoubleRow
CORES = 8
CH = 512                        # one matmul output (one PSUM bank)
GW = 2048                       # col chunk / exp width (4 banks)
NG = N // GW                    # 4 col chunks
NT = N // P                     # 64 row tiles
TEMP_SCALE = 2.0

# slots per chunk group and how many of them accumulate column sums
GROUP_SLOTS = [6, 6, 4, 4]      # per core
GROUP_NDIAG = [2, 2, 2, 2]      # leading rowsum-only slots per group
NSLOT = sum(GROUP_SLOTS)        # 20

# chunk -> (diag row-tiles, colsum row-tiles)
CHUNK_TILES = [
    (list(range(0, 16)), list(range(16, 48))),
    (list(range(16, 32)), list(range(32, 64))),
    (list(range(32, 48)), list(range(48, 64))),
    (list(range(48, 64)), list(range(0, 16))),
]


def _core_slots(c):
    """Row-tile id for each of core c's 20 slots, in program order."""
    out = []
    for q in range(NG):
        diag, off = CHUNK_TILES[q]
        nd = GROUP_NDIAG[q]
        nc_ = GROUP_SLOTS[q] - nd
        out.extend(diag[nd * c:nd * (c + 1)])
        out.extend(off[nc_ * c:nc_ * (c + 1)])
    return out


def _build_program():
    nc = bacc.Bacc(None, target_bir_lowering=False)

    lhs_d = nc.declare_dram_parameter("lhs", [P, KT, NSLOT * P], FP8,
                                      isOutput=False)
    dr_d = [nc.declare_dram_parameter(f"dr{q}", [P, KT, GW], FP8,
                                      isOutput=False) for q in range(NG)]
    out_d = nc.declare_dram_parameter("out", [P, NSLOT], F32, isOutput=True)
    acc_d = nc.declare_dram_parameter("acc", [P, NG * GW], F16, isOutput=True)

    Exp = mybir.ActivationFunctionType.Exp
    DR = mybir.MatmulPerfMode.DoubleRow
    add = mybir.AluOpType.add
    mult = mybir.AluOpType.mult

    with tile.TileContext(nc) as tc, ExitStack() as ctx:
        persist = ctx.enter_context(tc.tile_pool(name="persist", bufs=1))
        lhs_sb = persist.tile([P, KT, NSLOT * P], FP8, tag="lhs_sb",
                              name="lhs_sb")
        dr_sb = [persist.tile([P, KT, GW], FP8, tag=f"dr_sb{q}",
                              name=f"dr_sb{q}") for q in range(NG)]
        sall = persist.tile([P, NSLOT], F32, tag="sall", name="sall")
        accX = persist.tile([P, GW], F16, tag="accX", name="accX")
        accY = persist.tile([P, GW], F16, tag="accY", name="accY")
        nc.sync.dma_start(lhs_sb[:], lhs_d[:])
        for q in range(NG):
            nc.sync.dma_start(dr_sb[q][:], dr_d[q][:])

        with (
            tc.tile_pool(name="psm", bufs=2, space="PSUM") as psm,
            tc.tile_pool(name="junkp", bufs=3) as junkp,
        ):
            slot = 0
            for q in range(NG):
                qs = slice(q * GW, (q + 1) * GW)
                for k in range(GROUP_SLOTS[q]):
                    lhsT = lhs_sb[:, :, slot * P:(slot + 1) * P]
                    S4 = psm.tile([P, GW], F32, tag="S4")
                    for j in range(GW // CH):
                        nc.tensor.matmul(
                            S4[:, j * CH:(j + 1) * CH],
                            lhsT,
                            dr_sb[q][:, :, j * CH:(j + 1) * CH],
                            start=True, stop=True, perf_mode=DR,
                        )
                    E = junkp.tile([P, GW], F16, tag="E")
                    nc.scalar.activation(
                        E[:], S4[:], Exp, scale=TEMP_SCALE,
                        accum_out=sall[:, slot:slot + 1],
                    )
                    kk = k - GROUP_NDIAG[q]
                    if kk == 0:
                        nc.vector.tensor_copy(accX[:], E[:])
                    elif kk > 0:
                        src, dst = (accX, accY) if kk % 2 else (accY, accX)
                        nc.vector.scalar_tensor_tensor(
                            dst[:], src[:], 1.0, E[:], mult, add)
                    slot += 1
                # every group ends after an odd number of adds -> accY
                nc.sync.dma_start(acc_d[:, qs], accY[:])

        nc.sync.dma_start(out_d[:], sall[:])

    nc.compile()
    return nc


_NC_CACHE = []


def _get_nc():
    if not _NC_CACHE:
        _NC_CACHE.append(_build_program())
    return _NC_CACHE[0]


def _host_prep(emb_i, emb_j, tags, document_ids):
    emb = np.concatenate(
        [np.asarray(emb_i), np.asarray(emb_j)], axis=0).astype(np.float64)
    z = emb / np.linalg.norm(emb, axis=1, keepdims=True)
    z8 = z.astype(np.float32).astype(FP8NP)
    z8f = z8.astype(np.float32)

    dr = np.ascontiguousarray(z8.T.reshape(KT, P, N).transpose(1, 0, 2))

    slabs = {f"dr{q}": np.ascontiguousarray(dr[:, :, q * GW:(q + 1) * GW])
             for q in range(NG)}
    in_maps = []
    for c in range(CORES):
        tiles = _core_slots(c)
        lhs = np.concatenate(
            [dr[:, :, i * P:(i + 1) * P] for i in tiles], axis=2)
        in_maps.append({"lhs": np.ascontiguousarray(lhs), **slabs})

    tags2 = np.concatenate([tags, tags]).astype(np.int64)
    docs2 = np.concatenate([document_ids, document_ids]).astype(np.int64)
    corr = np.zeros(N, dtype=np.float64)
    for key, sign in ((tags2, 1.0), (docs2, 1.0),
                      (tags2 * 1024 + docs2, -1.0)):
        order = np.argsort(key, kind="stable")
        sk = key[order]
        starts = np.flatnonzero(np.r_[True, sk[1:] != sk[:-1]])
        bounds = np.r_[starts, len(sk)]
        for a, b in zip(bounds[:-1], bounds[1:]):
            idx = order[a:b]
            G = z8f[idx] @ z8f[idx].T
            corr[idx] += sign * np.exp(2.0 * G.astype(np.float64)).sum(1)

    zd = (z[:B] * z[B:]).sum(1)
    zdot = np.concatenate([zd, zd])
    return in_maps, corr, zdot


def _assemble_loss(results, corr, zdot):
    rowsum = np.zeros(N, dtype=np.float64)
    for c in range(CORES):
        o = np.asarray(results[c]["out"]).astype(np.float64)   # [128, 20]
        for s, i in enumerate(_core_slots(c)):
            rowsum[i * P:(i + 1) * P] += o[:, s]
        acc = np.asarray(results[c]["acc"]).astype(np.float64)  # [128, 8192]
        rowsum += acc.sum(0)
    denom = rowsum - corr + 0.1
    loss = (np.log(denom) - TEMP_SCALE * zdot).sum() / N
    return np.float32(loss)


def kernel(emb_i, emb_j, tags, num_classes, document_ids):
    nc = _get_nc()
    in_maps, corr, zdot = _host_prep(emb_i, emb_j, tags, document_ids)
    res = run_bass_kernel_spmd(nc, in_maps, list(range(CORES)))
    return _assemble_loss(res.results, corr, zdot)


# revision 7
# speedup vs baseline: 1.4137x; 1.4137x over previous
"""Trainium2 Bass kernel for nn_ContrastiveLoss — v3: symmetry-halved.

The similarity matrix is symmetric, so each off-diagonal superblock pair
(2048x2048 superblocks, 4x4 grid) is computed ONCE; the mirrored
contribution is recovered by column-summing the exp'd tiles.  Row sums
come free via the exp activation's accum_out; column sums are
elementwise-accumulated on the (otherwise idle) DVE into an SBUF f16
tile per column chunk, shipped to the host, and partition-reduced there.

Exp work: 8192*(8192+2048)/2 = 41.9M entries vs 67M direct (63%).

Static SPMD schedule (identical program, per-core data):
  chunk0 cols [   0:2048]: diag rows block0 (16 tiles) + off rows blocks
      1,2 (32 colsum tiles)  -> per core 2 diag + 4 colsum slots
  chunk1 cols [2048:4096]: diag block1 + off blocks 2,3  -> 2 + 4
  chunk2 cols [4096:6144]: diag block2 + off block 3     -> 2 + 2
  chunk3 cols [6144:8192]: diag block3 + off block 0     -> 2 + 2
Each pair {a,b} of superblocks is covered exactly once (rowsum covers
one orientation, colsum the mirror); diagonal superblocks are computed
in full and rowsum-only.  Per core: 20 tile-jobs, identical structure.

Host does normalization, fp8 cast, exact masked-pair corrections and
the numerator, as in v2.
"""

import sys

for _p in ("/opt/trn_rl_repo", "/root/.axon_site/_ro/trn_rl_repo"):
    if _p not in sys.path:
        sys.path.insert(0, _p)

from contextlib import ExitStack

import ml_dtypes
import numpy as np

from concourse import bacc, mybir, tile
from concourse.bass_utils import run_bass_kernel_spmd

F32 = mybir.dt.float32
F16 = mybir.dt.float16
FP8 = mybir.dt.float8e4
FP8NP = ml_dtypes.float8_e4m3

P = 128
B = 4096
D = 256
N = 2 * B                       # 8192
KT = D // P                     # 2 k-tiles, fused via DoubleRow
CORES = 8
CH = 512                        # one matmul output (one PSUM bank)
GW = 2048                       # col chunk / exp width (4 banks)
NG = N // GW                    # 4 col chunks
NT = N // P                     # 64 row tiles
TEMP_SCALE = 2.0

# slots per chunk group and how many of them accumulate column sums
GROUP_SLOTS = [6, 6, 4, 4]      # per core
GROUP_NDIAG = [2, 2, 2, 2]      # leading rowsum-only slots per group
NSLOT = sum(GROUP_SLOTS)        # 20

# chunk -> (diag row-tiles, colsum row-tiles)
CHUNK_TILES = [
    (list(range(0, 16)), list(range(16, 48))),
    (list(range(16, 32)), list(range(32, 64))),
    (list(range(32, 48)), list(range(48, 64))),
    (list(range(48, 64)), list(range(0, 16))),
]


def _core_slots(c):
    """Row-tile id for each of core c's 20 slots, in program order."""
    out = []
    for q in range(NG):
        diag, off = CHUNK_TILES[q]
        nd = GROUP_NDIAG[q]
        nc_ = GROUP_SLOTS[q] - nd
        out.extend(diag[nd * c:nd * (c + 1)])
        out.extend(off[nc_ * c:nc_ * (c + 1)])
    return out


def _build_program():
    nc = bacc.Bacc(None, target_bir_lowering=False)

    lhs_d = nc.declare_dram_parameter("lhs", [P, KT, NSLOT * P], FP8,
                                      isOutput=False)
    dr_d = [nc.declare_dram_parameter(f"dr{q}", [P, KT, GW], FP8,
                                      isOutput=False) for q in range(NG)]
    out_d = nc.declare_dram_parameter("out", [P, NSLOT], F32, isOutput=True)
    acc_d = nc.declare_dram_parameter("acc", [P, NG * GW], F16, isOutput=True)

    Exp = mybir.ActivationFunctionType.Exp
    DR = mybir.MatmulPerfMode.DoubleRow
    add = mybir.AluOpType.add
    mult = mybir.AluOpType.mult

    NLA = GROUP_SLOTS[0]            # slots served by the small lhs tile

    with tile.TileContext(nc) as tc, ExitStack() as ctx:
        persist = ctx.enter_context(tc.tile_pool(name="persist", bufs=1))
        lhs_sbA = persist.tile([P, KT, NLA * P], FP8, tag="lhs_sbA",
                               name="lhs_sbA")
        lhs_sbB = persist.tile([P, KT, (NSLOT - NLA) * P], FP8,
                               tag="lhs_sbB", name="lhs_sbB")
        dr0_sb = [persist.tile([P, KT, CH], FP8, tag=f"dr0_sb{j}",
                               name=f"dr0_sb{j}") for j in range(GW // CH)]
        dr_sb = [None] + [persist.tile([P, KT, GW], FP8, tag=f"dr_sb{q}",
                                       name=f"dr_sb{q}")
                          for q in range(1, NG)]
        sall = persist.tile([P, NSLOT], F32, tag="sall", name="sall")
        accX = persist.tile([P, GW], F16, tag="accX", name="accX")
        accY = persist.tile([P, GW], F16, tag="accY", name="accY")

        # issue order = arrival order: everything the first matmuls need
        # first, the rest behind it
        nc.sync.dma_start(lhs_sbA[:], lhs_d[:, :, :NLA * P])
        for j in range(GW // CH):
            nc.sync.dma_start(dr0_sb[j][:], dr_d[0][:, :, j * CH:(j + 1) * CH])
        nc.sync.dma_start(lhs_sbB[:], lhs_d[:, :, NLA * P:])
        for q in range(1, NG):
            nc.sync.dma_start(dr_sb[q][:], dr_d[q][:])

        def lhsT_of(slot):
            if slot < NLA:
                return lhs_sbA[:, :, slot * P:(slot + 1) * P]
            s = slot - NLA
            return lhs_sbB[:, :, s * P:(s + 1) * P]

        def rhs_of(q, j):
            if q == 0:
                return dr0_sb[j][:]
            return dr_sb[q][:, :, j * CH:(j + 1) * CH]

        with (
            tc.tile_pool(name="psm", bufs=2, space="PSUM") as psm,
            tc.tile_pool(name="junkp", bufs=3) as junkp,
        ):
            slot = 0
            for q in range(NG):
                qs = slice(q * GW, (q + 1) * GW)
                for k in range(GROUP_SLOTS[q]):
                    lhsT = lhsT_of(slot)
                    S4 = psm.tile([P, GW], F32, tag="S4")
                    for j in range(GW // CH):
                        nc.tensor.matmul(
                            S4[:, j * CH:(j + 1) * CH],
                            lhsT,
                            rhs_of(q, j),
                            start=True, stop=True, perf_mode=DR,
                        )
                    E = junkp.tile([P, GW], F16, tag="E")
                    nc.scalar.activation(
                        E[:], S4[:], Exp, scale=TEMP_SCALE,
                        accum_out=sall[:, slot:slot + 1],
                    )
                    kk = k - GROUP_NDIAG[q]
                    if kk == 0:
                        nc.vector.tensor_copy(accX[:], E[:])
                    elif kk > 0:
                        src, dst = (accX, accY) if kk % 2 else (accY, accX)
                        nc.vector.scalar_tensor_tensor(
                            dst[:], src[:], 1.0, E[:], mult, add)
                    slot += 1
                # every group ends after an odd number of adds -> accY
                nc.sync.dma_start(acc_d[:, qs], accY[:])

        nc.sync.dma_start(out_d[:], sall[:])

    nc.compile()
    return nc


_NC_CACHE = []


def _get_nc():
    if not _NC_CACHE:
        _NC_CACHE.append(_build_program())
    return _NC_CACHE[0]


def _host_prep(emb_i, emb_j, tags, document_ids):
    emb = np.concatenate(
        [np.asarray(emb_i), np.asarray(emb_j)], axis=0).astype(np.float64)
    z = emb / np.linalg.norm(emb, axis=1, keepdims=True)
    z8 = z.astype(np.float32).astype(FP8NP)
    z8f = z8.astype(np.float32)

    dr = np.ascontiguousarray(z8.T.reshape(KT, P, N).transpose(1, 0, 2))

    slabs = {f"dr{q}": np.ascontiguousarray(dr[:, :, q * GW:(q + 1) * GW])
             for q in range(NG)}
    in_maps = []
    for c in range(CORES):
        tiles = _core_slots(c)
        lhs = np.concatenate(
            [dr[:, :, i * P:(i + 1) * P] for i in tiles], axis=2)
        in_maps.append({"lhs": np.ascontiguousarray(lhs), **slabs})

    tags2 = np.concatenate([tags, tags]).astype(np.int64)
    docs2 = np.concatenate([document_ids, document_ids]).astype(np.int64)
    corr = np.zeros(N, dtype=np.float64)
    for key, sign in ((tags2, 1.0), (docs2, 1.0),
                      (tags2 * 1024 + docs2, -1.0)):
        order = np.argsort(key, kind="stable")
        sk = key[order]
        starts = np.flatnonzero(np.r_[True, sk[1:] != sk[:-1]])
        bounds = np.r_[starts, len(sk)]
        for a, b in zip(bounds[:-1], bounds[1:]):
            idx = order[a:b]
            G = z8f[idx] @ z8f[idx].T
            corr[idx] += sign * np.exp(2.0 * G.astype(np.float64)).sum(1)

    zd = (z[:B] * z[B:]).sum(1)
    zdot = np.concatenate([zd, zd])
    return in_maps, corr, zdot


def _assemble_loss(results, corr, zdot):
    rowsum = np.zeros(N, dtype=np.float64)
    for c in range(CORES):
        o = np.asarray(results[c]["out"]).astype(np.float64)   # [128, 20]
        for s, i in enumerate(_core_slots(c)):
            rowsum[i * P:(i + 1) * P] += o[:, s]
        acc = np.asarray(results[c]["acc"]).astype(np.float64)  # [128, 8192]
        rowsum += acc.sum(0)
    denom = rowsum - corr + 0.1
    loss = (np.log(denom) - TEMP_SCALE * zdot).sum() / N
    return np.float32(loss)


def kernel(emb_i, emb_j, tags, num_classes, document_ids):
    nc = _get_nc()
    in_maps, corr, zdot = _host_prep(emb_i, emb_j, tags, document_ids)
    res = run_bass_kernel_spmd(nc, in_maps, list(range(CORES)))
    return _assemble_loss(res.results, corr, zdot)
